# revision 1
# baseline (speedup 1.0000x reference)
"""CopyTokenDecoder Trainium2 kernel.

Sharding: data-parallel over batch B=8 -> one NeuronCore per batch element.
Each core runs the full per-batch pipeline: single-head attention front-end,
gating, FFN, the [T,D]x[D,V] output projection with local softmax over the
full vocab, the copy-mechanism scatter-add (realized as small matmuls against
a host-built routing one-hot), and the final log.

Per-core layouts (P = 128 partitions):
  feature-major  [d_lo(128), d_hi, t]   for matmul operands
  token-major    [t_lo(128), t_hi, d]   for layernorms / row-wise scaling
The s (memory) axis is host-side sorted by copy-token vocab bucket and padded
to a 64-window x WSLOT slot grid so the scatter becomes, per 512-wide vocab
tile, one K=WSLOT matmul (exp_scores-window x one-hot window). Padded slots
are masked to -1e30 pre-softmax so they contribute exp()=0 everywhere.
"""

from contextlib import ExitStack

import numpy as np
import ml_dtypes

import concourse.tile as tile
from concourse import bacc, mybir
from concourse.bass_utils import run_bass_kernel_spmd
from concourse.masks import make_identity

F32 = mybir.dt.float32
BF16 = mybir.dt.bfloat16
AF = mybir.ActivationFunctionType
OP = mybir.AluOpType
BF = ml_dtypes.bfloat16

T, B, S, D, F, V = 256, 8, 512, 512, 2048, 32000
P = 128
DSCALE = float(D) ** -0.5
NEG = -1.0e30
TT = 2                      # t-tiles of 128
NWIN = 63                   # 512-wide vocab windows (last covers 256)
NCHUNK = 16                 # vocab chunks of 2048 (last 1280)
CHUNK = 2048
EPS_LN = 1e-5
EPS_LOG = 1e-12

_CACHE = {}


def _subwidths(c):
    if c < NCHUNK - 1:
        return [512, 512, 512, 512]
    return [512, 512, 256]


def _build(wslot):
    sp = 64 * wslot           # padded slot count (s' axis)
    nhi = sp // P             # s'-outer size
    wpb = P // wslot          # windows per 128-partition block

    nc = bacc.Bacc("TRN2", target_bir_lowering=False, debug=False,
                   enable_asserts=False, num_devices=B)

    def din(name, shape, dt):
        return nc.dram_tensor(name, shape, dt, kind="ExternalInput").ap()

    # per-core tensors
    outsT_d = din("outsT", [D, T], BF16)
    outs_tok_d = din("outs_tok", [T, D], F32)
    memT_d = din("memT", [D, S], BF16)
    maskrow_d = din("maskrow", [1, S], BF16)
    pmat_d = din("pmat", [S, sp], BF16)
    onehot_d = din("onehot", [P, nhi * 512], BF16)
    # shared weights
    wqT_d = din("wqT", [D, D], BF16)
    wkT_d = din("wkT", [D, D], BF16)
    wvT_d = din("wvT", [D, D], BF16)
    woT_d = din("woT", [D, D], BF16)
    w1T_d = din("w1T", [D, F], BF16)
    w2T_d = din("w2T", [F, D], BF16)
    wembW_d = din("wembW", [NWIN, P, 4 * 512], BF16)
    bq_d = din("bq_c", [P, 4], F32)
    bk_d = din("bk_c", [P, 4], F32)
    bvrow_d = din("bv_row", [1, D], BF16)
    bo_tok_d = din("bo_tok", [P, D], F32)
    b1_d = din("b1_c", [P, 16], F32)
    b2_d = din("b2_c", [P, 4], F32)
    g1_d = din("g1_tok", [P, D], F32)
    b1g_d = din("b1g_tok", [P, D], F32)
    g2_d = din("g2_tok", [P, D], F32)
    b2g_d = din("b2g_tok", [P, D], F32)
    wdd_d = din("wd_diff_tok", [P, 2 * D], F32)
    bdd_d = din("bddiff", [P, 1], F32)
    ones_d = din("ones_row", [1, T], BF16)

    out_d = nc.dram_tensor("out", [T, V], F32, kind="ExternalOutput").ap()
    out_r = out_d.rearrange("(th tl) v -> tl th v", tl=P)

    r3 = lambda ap, inner: ap.rearrange("(hi lo) x -> lo hi x", lo=P)

    with tile.TileContext(nc) as tc, ExitStack() as octx:
        cpool = octx.enter_context(tc.tile_pool(name="cpool", bufs=1))
        # ---- persistent tiles (live through pass A/B) ----
        onehot = cpool.tile([P, nhi, 512], BF16, tag="onehot")
        nc.sync.dma_start(onehot[:], onehot_d.rearrange("p (hi v) -> p hi v", v=512))
        exp_st = cpool.tile([P, nhi, T], BF16, tag="exp_st")
        x2T = cpool.tile([P, 4, T], BF16, tag="x2T")
        ident_f = cpool.tile([P, P], F32, tag="ident_f")
        make_identity(nc, ident_f[:])
        ident_b = cpool.tile([P, P], BF16, tag="ident_b")
        nc.vector.tensor_copy(ident_b[:], ident_f[:])
        rr = cpool.tile([P, TT], F32, tag="rr")          # 1/sum_s exp(scores)
        cg = cpool.tile([P, TT], F32, tag="cg")          # copy gate
        gg = cpool.tile([P, TT], F32, tag="gg")          # gen gate
        sv_parts = cpool.tile([P, 2 * NCHUNK], F32, tag="sv_parts")
        svs = cpool.tile([P, TT], F32, tag="svs")
        beta = cpool.tile([P, TT], F32, tag="beta")      # bf16-rounded, as f32
        slog = cpool.tile([P, TT], F32, tag="slog")      # final log scale
        onesr = cpool.tile([1, T], BF16, tag="onesr")
        nc.sync.dma_start(onesr[:], ones_d[:])
        maskrow = cpool.tile([1, S], BF16, tag="maskrow")
        nc.sync.dma_start(maskrow[:], maskrow_d[:])
        eps_ln_c = cpool.tile([P, 1], F32, tag="eps_ln_c")
        nc.gpsimd.memset(eps_ln_c[:], EPS_LN)
        eps_log_c = cpool.tile([P, 1], F32, tag="eps_log_c")
        nc.gpsimd.memset(eps_log_c[:], EPS_LOG)

        # ================= front-end (staged scoped pools) =================
        fctx = ExitStack()
        fe = fctx.enter_context(tc.tile_pool(name="fe", bufs=1))

        def load(pool, dram_ap, shape, tag):
            t_ = pool.tile(shape, dram_ap.dtype, tag=tag, name=tag)
            nc.sync.dma_start(t_[:], dram_ap)
            return t_

        # persists across both front-end stages
        outs_tok = load(fe, outs_tok_d.rearrange("(th tl) d -> tl th d", tl=P),
                        [P, TT, D], "outs_tok")
        bo_tok = load(fe, bo_tok_d, [P, D], "bo_tok")
        g1_tok = load(fe, g1_d, [P, D], "g1_tok")
        b1g_tok = load(fe, b1g_d, [P, D], "b1g_tok")
        g2_tok = load(fe, g2_d, [P, D], "g2_tok")
        b2g_tok = load(fe, b2g_d, [P, D], "b2g_tok")
        wd_diff = load(fe, wdd_d, [P, 2 * D], "wd_diff")
        bddiff = load(fe, bdd_d, [P, 1], "bddiff")
        attn_tok = fe.tile([P, TT, D], F32, tag="attn_tok")

        def layer_norm(pool, scr_pool, dst, src_ap, g_t, b_t, nm):
            """dst[:] = LN(src_ap) * g + b   (token-major [P, D] slices)"""
            mu = pool.tile([P, 1], F32, tag=f"mu_{nm}", name=f"mu_{nm}")
            nc.vector.reduce_sum(mu[:], src_ap, axis=mybir.AxisListType.X)
            nc.vector.tensor_scalar(out=mu[:], in0=mu[:], scalar1=1.0 / D,
                                    scalar2=None, op0=OP.mult)
            xc = scr_pool.tile([P, D], F32, tag="ln_xc", name="ln_xc")
            nc.vector.tensor_scalar(out=xc[:], in0=src_ap, scalar1=mu[:],
                                    scalar2=None, op0=OP.subtract)
            scr = scr_pool.tile([P, D], F32, tag="ln_scr", name="ln_scr")
            ss = pool.tile([P, 1], F32, tag=f"ss_{nm}", name=f"ss_{nm}")
            nc.scalar.activation(scr[:], xc[:], AF.Square, accum_out=ss[:])
            std = pool.tile([P, 1], F32, tag=f"std_{nm}", name=f"std_{nm}")
            nc.scalar.activation(std[:], ss[:], AF.Sqrt, bias=eps_ln_c[:, :1],
                                 scale=1.0 / D)
            rstd = pool.tile([P, 1], F32, tag=f"rstd_{nm}", name=f"rstd_{nm}")
            nc.vector.reciprocal(rstd[:], std[:])
            nc.vector.scalar_tensor_tensor(out=dst, in0=xc[:], scalar=rstd[:],
                                           in1=g_t[:], op0=OP.mult, op1=OP.mult)
            nc.vector.tensor_add(dst, dst, b_t[:])

        # ---------------- stage A: attention ----------------
        with ExitStack() as actx:
            fa = actx.enter_context(tc.tile_pool(name="fa", bufs=1))
            fad = actx.enter_context(tc.tile_pool(name="fad", bufs=2))
            fp = actx.enter_context(tc.tile_pool(name="fp", bufs=6, space="PSUM"))
            fp5 = actx.enter_context(tc.tile_pool(name="fp5", bufs=2, space="PSUM"))

            outsT = load(fa, r3(outsT_d, T), [P, 4, T], "outsT")
            memT = load(fa, r3(memT_d, S), [P, 4, S], "memT")
            pmat = load(fa, r3(pmat_d, sp), [P, 4, sp], "pmat")
            wqT = load(fa, r3(wqT_d, D), [P, 4, D], "wqT")
            wkT = load(fa, r3(wkT_d, D), [P, 4, D], "wkT")
            wvT = load(fa, r3(wvT_d, D), [P, 4, D], "wvT")
            woT = load(fa, r3(woT_d, D), [P, 4, D], "woT")
            bq_c = load(fa, bq_d, [P, 4], "bq_c")
            bk_c = load(fa, bk_d, [P, 4], "bk_c")
            bv_row = load(fa, bvrow_d, [1, D], "bv_row")

            # PE warm-up: ~4us of junk matmuls gated only on on-chip data
            # (identity/memset) so they run while the input DMAs land and the
            # HAM clock-gate reaches 8/8 before the real matmuls start.
            wu = fa.tile([P, 512], BF16, tag="wu")
            nc.gpsimd.memset(wu[:], 0.0)
            wu_ps = fp5.tile([P, 512], F32, tag="ps512", space="PSUM")
            for i in range(20):
                nc.tensor.matmul(wu_ps[:], lhsT=ident_b[:], rhs=wu[:],
                                 start=(i == 0), stop=(i == 19))

            # q/k projections (feature-major, s compact & host-sorted)
            qT = fa.tile([P, 4, T], BF16, tag="qT")
            for ho in range(4):
                ps = fp.tile([P, T], F32, tag="ps256", space="PSUM")
                for k in range(4):
                    nc.tensor.matmul(ps[:], lhsT=wqT[:, k, ho * P:(ho + 1) * P],
                                     rhs=outsT[:, k, :], start=(k == 0),
                                     stop=(k == 3))
                # q = (outs@Wq.T + bq) * DSCALE
                nc.vector.tensor_scalar(out=qT[:, ho, :], in0=ps[:],
                                        scalar1=bq_c[:, ho:ho + 1],
                                        scalar2=DSCALE, op0=OP.add, op1=OP.mult)
            kT = fa.tile([P, 4, S], BF16, tag="kT")
            for ho in range(4):
                ps = fp5.tile([P, 512], F32, tag="ps512", space="PSUM")
                for k in range(4):
                    nc.tensor.matmul(ps[:], lhsT=wkT[:, k, ho * P:(ho + 1) * P],
                                     rhs=memT[:, k, :], start=(k == 0),
                                     stop=(k == 3))
                nc.vector.tensor_scalar(out=kT[:, ho, :], in0=ps[:],
                                        scalar1=bk_c[:, ho:ho + 1],
                                        scalar2=None, op0=OP.add)
            # v (s-major)
            v_sb = fa.tile([P, 4, D], BF16, tag="v_sb")
            for sc in range(4):
                ps = fp5.tile([P, 512], F32, tag="ps512", space="PSUM")
                for k in range(4):
                    nc.tensor.matmul(ps[:], lhsT=memT[:, k, sc * P:(sc + 1) * P],
                                     rhs=wvT[:, k, :], start=(k == 0), stop=False)
                nc.tensor.matmul(ps[:], lhsT=onesr[:1, :P], rhs=bv_row[:],
                                 start=False, stop=True)
                if sc % 2 == 0:
                    nc.vector.tensor_copy(v_sb[:, sc, :], ps[:])
                else:
                    nc.scalar.copy(v_sb[:, sc, :], ps[:])

            # scoresT -> exp_c (s-major, compact)
            exp_c = fa.tile([P, 4, T], BF16, tag="exp_c")
            for sc in range(4):
                ps = fp.tile([P, T], F32, tag="ps256", space="PSUM")
                for k in range(4):
                    nc.tensor.matmul(ps[:], lhsT=kT[:, k, sc * P:(sc + 1) * P],
                                     rhs=qT[:, k, :], start=(k == 0), stop=False)
                nc.tensor.matmul(ps[:], lhsT=maskrow[:1, sc * P:(sc + 1) * P],
                                 rhs=onesr[:1, :], start=False, stop=True)
                nc.scalar.activation(exp_c[:, sc, :], ps[:], AF.Exp)

            # permute/pad exp_c into the slot grid used by the scatter pass:
            # exp_st[slot, t] = sum_i pmat[i, slot] * exp_c[i, t]
            # (pad slots have all-zero pmat columns -> exact zeros)
            for so in range(nhi):
                ps = fp.tile([P, T], F32, tag="ps256", space="PSUM")
                for k in range(4):
                    nc.tensor.matmul(ps[:], lhsT=pmat[:, k, so * P:(so + 1) * P],
                                     rhs=exp_c[:, k, :], start=(k == 0),
                                     stop=(k == 3))
                if so % 2 == 0:
                    nc.vector.tensor_copy(exp_st[:, so, :], ps[:])
                else:
                    nc.scalar.copy(exp_st[:, so, :], ps[:])

            # scores token-major: only for attention softmax row-sums
            ratt_parts = fa.tile([P, TT], F32, tag="ratt_parts")
            for tt in range(TT):
                ps = fp5.tile([P, 512], F32, tag="ps512", space="PSUM")
                for k in range(4):
                    nc.tensor.matmul(ps[:], lhsT=qT[:, k, tt * P:(tt + 1) * P],
                                     rhs=kT[:, k, :], start=(k == 0), stop=False)
                nc.tensor.matmul(ps[:], lhsT=onesr[:1, :P], rhs=maskrow[:1, :],
                                 start=False, stop=True)
                scr = fad.tile([P, 512], F32, tag="scr_ts", name="scr_ts")
                nc.scalar.activation(scr[:], ps[:], AF.Exp,
                                     accum_out=ratt_parts[:, tt:tt + 1])
            nc.vector.reciprocal(rr[:], ratt_parts[:])

            # attention value mix + output projection (feature-major)
            attnT = fa.tile([P, 4, T], BF16, tag="attnT")
            for dc in range(4):
                ps = fp.tile([P, T], F32, tag="ps256", space="PSUM")
                for sc in range(4):
                    nc.tensor.matmul(ps[:], lhsT=v_sb[:, sc, dc * P:(dc + 1) * P],
                                     rhs=exp_c[:, sc, :], start=(sc == 0),
                                     stop=(sc == 3))
                if dc % 2 == 0:
                    nc.vector.tensor_copy(attnT[:, dc, :], ps[:])
                else:
                    nc.scalar.copy(attnT[:, dc, :], ps[:])
            attn_oT = fa.tile([P, 4, T], F32, tag="attn_oT")
            for ho in range(4):
                ps = fp.tile([P, T], F32, tag="ps256", space="PSUM")
                for k in range(4):
                    nc.tensor.matmul(ps[:], lhsT=woT[:, k, ho * P:(ho + 1) * P],
                                     rhs=attnT[:, k, :], start=(k == 0),
                                     stop=(k == 3))
                if ho % 2 == 0:
                    nc.vector.tensor_copy(attn_oT[:, ho, :], ps[:])
                else:
                    nc.scalar.copy(attn_oT[:, ho, :], ps[:])

            # transpose to token-major; normalize rows; add bo
            for tt in range(TT):
                for ho in range(4):
                    pst = fp.tile([P, T], F32, tag="ps256", space="PSUM")
                    nc.tensor.transpose(pst[:, :P],
                                        attn_oT[:, ho, tt * P:(tt + 1) * P],
                                        ident_f[:])
                    if ho % 2 == 0:
                        nc.vector.tensor_copy(
                            attn_tok[:, tt, ho * P:(ho + 1) * P], pst[:, :P])
                    else:
                        nc.scalar.copy(attn_tok[:, tt, ho * P:(ho + 1) * P],
                                       pst[:, :P])
            for tt in range(TT):
                nc.vector.scalar_tensor_tensor(
                    out=attn_tok[:, tt, :], in0=attn_tok[:, tt, :],
                    scalar=rr[:, tt:tt + 1], in1=bo_tok[:],
                    op0=OP.mult, op1=OP.add)

        # ---------------- stage B: gates + FFN ----------------
        with ExitStack() as bctx:
            fb = bctx.enter_context(tc.tile_pool(name="fb", bufs=1))
            fbd = bctx.enter_context(tc.tile_pool(name="fbd", bufs=2))
            fp = bctx.enter_context(tc.tile_pool(name="fp2", bufs=8, space="PSUM"))

            w1T = load(fb, r3(w1T_d, F), [P, 4, F], "w1T")
            w2T = load(fb, r3(w2T_d, D), [P, 16, D], "w2T")
            b1_c = load(fb, b1_d, [P, 16], "b1_c")
            b2_c = load(fb, b2_d, [P, 4], "b2_c")

            # gates from [outs ; LN(attn)] (softmax2 == sigmoid of logit diff)
            attn_n = fbd.tile([P, D], F32, tag="attn_n", name="attn_n")
            ld = fb.tile([P, TT], F32, tag="ld")
            for tt in range(TT):
                layer_norm(fb, fbd, attn_n[:], attn_tok[:, tt, :], g1_tok,
                           b1g_tok, "an")
                lda = fb.tile([P, 1], F32, tag="lda")
                ldb = fb.tile([P, 1], F32, tag="ldb")
                scr = fbd.tile([P, D], F32, tag="ld_scr", name="ld_scr")
                nc.vector.scalar_tensor_tensor(out=scr[:], in0=outs_tok[:, tt, :],
                                               scalar=1.0, in1=wd_diff[:, :D],
                                               op0=OP.mult, op1=OP.mult,
                                               accum_out=lda[:])
                scr2 = fbd.tile([P, D], F32, tag="ld_scr2", name="ld_scr2")
                nc.vector.scalar_tensor_tensor(out=scr2[:], in0=attn_n[:],
                                               scalar=1.0, in1=wd_diff[:, D:],
                                               op0=OP.mult, op1=OP.mult,
                                               accum_out=ldb[:])
                nc.vector.tensor_add(ld[:, tt:tt + 1], lda[:], ldb[:])
            nc.scalar.activation(cg[:], ld[:], AF.Sigmoid, bias=bddiff[:, :1])
            nc.vector.tensor_scalar(out=gg[:], in0=cg[:], scalar1=-1.0,
                                    scalar2=1.0, op0=OP.mult, op1=OP.add)

            # residual + LN1 -> x ; FFN ; LN2 -> x2 ; transpose -> x2T
            x_tok = fb.tile([P, TT, D], F32, tag="x_tok")
            for tt in range(TT):
                res = fbd.tile([P, D], F32, tag="res", name="res")
                nc.vector.tensor_add(res[:], outs_tok[:, tt, :],
                                     attn_tok[:, tt, :])
                layer_norm(fb, fbd, x_tok[:, tt, :], res[:], g1_tok, b1g_tok, "x")
            xT = fb.tile([P, 4, T], BF16, tag="xT")
            for tt in range(TT):
                for k in range(4):
                    pst = fp.tile([P, T], F32, tag="ps256", space="PSUM")
                    nc.tensor.transpose(pst[:, :P],
                                        x_tok[:, tt, k * P:(k + 1) * P],
                                        ident_f[:])
                    if k % 2 == 0:
                        nc.vector.tensor_copy(xT[:, k, tt * P:(tt + 1) * P],
                                              pst[:, :P])
                    else:
                        nc.scalar.copy(xT[:, k, tt * P:(tt + 1) * P],
                                       pst[:, :P])
            h1T = fb.tile([P, 16, T], BF16, tag="h1T")
            for fc in range(16):
                ps = fp.tile([P, T], F32, tag="ps256", space="PSUM")
                for k in range(4):
                    nc.tensor.matmul(ps[:], lhsT=w1T[:, k, fc * P:(fc + 1) * P],
                                     rhs=xT[:, k, :], start=(k == 0),
                                     stop=(k == 3))
                # relu(psum + b1)
                nc.vector.tensor_scalar(out=h1T[:, fc, :], in0=ps[:],
                                        scalar1=b1_c[:, fc:fc + 1], scalar2=0.0,
                                        op0=OP.add, op1=OP.max)
            hT = fb.tile([P, 4, T], F32, tag="hT")
            for ho in range(4):
                ps = fp.tile([P, T], F32, tag="ps256", space="PSUM")
                for fc in range(16):
                    nc.tensor.matmul(ps[:], lhsT=w2T[:, fc, ho * P:(ho + 1) * P],
                                     rhs=h1T[:, fc, :], start=(fc == 0),
                                     stop=(fc == 15))
                nc.vector.tensor_scalar(out=hT[:, ho, :], in0=ps[:],
                                        scalar1=b2_c[:, ho:ho + 1], scalar2=None,
                                        op0=OP.add)
            h_tok = fb.tile([P, TT, D], F32, tag="h_tok")
            for tt in range(TT):
                for ho in range(4):
                    pst = fp.tile([P, T], F32, tag="ps256", space="PSUM")
                    nc.tensor.transpose(pst[:, :P],
                                        hT[:, ho, tt * P:(tt + 1) * P],
                                        ident_f[:])
                    if ho % 2 == 0:
                        nc.vector.tensor_copy(
                            h_tok[:, tt, ho * P:(ho + 1) * P], pst[:, :P])
                    else:
                        nc.scalar.copy(h_tok[:, tt, ho * P:(ho + 1) * P],
                                       pst[:, :P])
            x2_tok = fb.tile([P, TT, D], F32, tag="x2_tok")
            for tt in range(TT):
                layer_norm(fb, fbd, x2_tok[:, tt, :], h_tok[:, tt, :], g2_tok,
                           b2g_tok, "x2")
            for tt in range(TT):
                for k in range(4):
                    pst = fp.tile([P, T], F32, tag="ps256", space="PSUM")
                    nc.tensor.transpose(pst[:, :P],
                                        x2_tok[:, tt, k * P:(k + 1) * P],
                                        ident_f[:])
                    if k % 2 == 0:
                        nc.vector.tensor_copy(x2T[:, k, tt * P:(tt + 1) * P],
                                              pst[:, :P])
                    else:
                        nc.scalar.copy(x2T[:, k, tt * P:(tt + 1) * P],
                                       pst[:, :P])

        fctx.close()

        # ================= vocab passes =================
        bigp = octx.enter_context(tc.tile_pool(name="bigp", bufs=1))
        strm_bufs, outp_bufs = (7, 3) if wslot <= 32 else (3, 2)
        strm = octx.enter_context(tc.tile_pool(name="strm", bufs=strm_bufs))
        outp = octx.enter_context(tc.tile_pool(name="outp", bufs=outp_bufs))
        mp = octx.enter_context(tc.tile_pool(name="mp", bufs=2, space="PSUM"))

        e_tiles = []
        for tt in range(TT):
            et_ = bigp.tile([P, V], BF16, tag=f"E{tt}", name=f"E{tt}")
            e_tiles.append(et_)

        # ---- pass A: logits -> exp -> E (bf16) + per-chunk row sums ----
        for c in range(NCHUNK):
            subws = _subwidths(c)
            subs = []
            for k_sub, wk in enumerate(subws):
                wflat = strm.tile([P, 4 * 512], BF16, tag="wemb_sub",
                                  name="wflat")
                w = 4 * c + k_sub
                nc.sync.dma_start(wflat[:], wembW_d[w])
                subs.append(wflat.rearrange("p (hi v) -> p hi v", v=512))
            for tt in range(TT):
                ps = mp.tile([P, CHUNK], F32, tag="bigps", space="PSUM")
                for k_sub, wk in enumerate(subws):
                    for k in range(4):
                        nc.tensor.matmul(
                            ps[:, k_sub * 512:k_sub * 512 + wk],
                            lhsT=x2T[:, k, tt * P:(tt + 1) * P],
                            rhs=subs[k_sub][:, k, :wk],
                            start=(k == 0), stop=(k == 3))
                cw = sum(subws)
                c0 = c * CHUNK
                nc.scalar.activation(
                    e_tiles[tt][:, c0:c0 + cw], ps[:, :cw], AF.Exp,
                    accum_out=sv_parts[:, tt * NCHUNK + c:tt * NCHUNK + c + 1])

        # ---- softmax denominator, scales, diag(beta) ----
        for tt in range(TT):
            nc.vector.reduce_sum(svs[:, tt:tt + 1],
                                 sv_parts[:, tt * NCHUNK:(tt + 1) * NCHUNK],
                                 axis=mybir.AxisListType.X)
        # beta = gg / (cg * rr * sv)  (rounded through bf16 so the diag matmul
        # uses the exact same value); slog = gg / (sv * beta_bf16)
        tmp = cpool.tile([P, TT], F32, tag="btmp")
        nc.vector.tensor_mul(tmp[:], cg[:], rr[:])
        nc.vector.tensor_mul(tmp[:], tmp[:], svs[:])
        rtmp = cpool.tile([P, TT], F32, tag="brec")
        nc.vector.reciprocal(rtmp[:], tmp[:])
        nc.vector.tensor_mul(rtmp[:], rtmp[:], gg[:])
        beta_b = cpool.tile([P, TT], BF16, tag="beta_b")
        nc.vector.tensor_copy(beta_b[:], rtmp[:])
        nc.vector.tensor_copy(beta[:], beta_b[:])          # bf16-rounded, f32
        nc.vector.tensor_mul(tmp[:], svs[:], beta[:])
        nc.vector.reciprocal(rtmp[:], tmp[:])
        nc.vector.tensor_mul(slog[:], rtmp[:], gg[:])
        diags = []
        for tt in range(TT):
            dg = cpool.tile([P, P], BF16, tag=f"diag{tt}", name=f"diag{tt}")
            nc.vector.tensor_scalar(out=dg[:], in0=ident_b[:],
                                    scalar1=beta[:, tt:tt + 1], scalar2=None,
                                    op0=OP.mult)
            diags.append(dg)

        # ---- pass B: psum = copy-delta + beta*E ; out = Ln(slog*psum + eps) ----
        for tt in range(TT):
            for c in range(NCHUNK):
                subws = _subwidths(c)
                ps = mp.tile([P, CHUNK], F32, tag="bigps", space="PSUM")
                for k_sub, wk in enumerate(subws):
                    w = 4 * c + k_sub                      # vocab window index
                    po = wslot * (w % wpb)
                    hi = w // wpb
                    pslice = ps[:, k_sub * 512:k_sub * 512 + wk]
                    nc.tensor.matmul(
                        pslice, lhsT=exp_st[po:po + wslot, hi, tt * P:(tt + 1) * P],
                        rhs=onehot[po:po + wslot, hi, :wk],
                        start=True, stop=False, tile_position=(po, 0))
                    nc.tensor.matmul(
                        pslice, lhsT=diags[tt][:],
                        rhs=e_tiles[tt][:, c * CHUNK + k_sub * 512:
                                        c * CHUNK + k_sub * 512 + wk],
                        start=False, stop=True, tile_position=(0, 0))
                cw = sum(subws)
                ot = outp.tile([P, CHUNK], F32, tag="out_sb")
                nc.scalar.activation(ot[:, :cw], ps[:, :cw], AF.Ln,
                                     bias=eps_log_c[:, :1],
                                     scale=slog[:, tt:tt + 1])
                nc.sync.dma_start(out_r[:, tt, c * CHUNK:c * CHUNK + cw],
                                  ot[:, :cw])

    nc.compile()
    return nc


def _tile_wemb(w_emb):
    wp = np.zeros((NWIN * 512, D), BF)
    wp[:V] = w_emb.astype(BF)
    # [w, v, hi, lo] -> [w, lo, hi, v] so each window is per-partition contiguous
    wt = wp.reshape(NWIN, 512, 4, P).transpose(0, 3, 2, 1)
    return np.ascontiguousarray(wt.reshape(NWIN, P, 4 * 512))


def _prep(inputs):
    g = {k: np.asarray(v) for k, v in inputs.items()}
    f32 = np.float32

    shared = {
        "wqT": np.ascontiguousarray(g["Wq"].T.astype(BF)),
        "wkT": np.ascontiguousarray(g["Wk"].T.astype(BF)),
        "wvT": np.ascontiguousarray(g["Wv"].T.astype(BF)),
        "woT": np.ascontiguousarray(g["Wo"].T.astype(BF)),
        "w1T": np.ascontiguousarray(g["W1"].T.astype(BF)),
        "w2T": np.ascontiguousarray(g["W2"].T.astype(BF)),
        "wembW": _tile_wemb(g["W_emb"]),
        "bq_c": np.ascontiguousarray(g["bq"].astype(f32).reshape(4, P).T),
        "bk_c": np.ascontiguousarray(g["bk"].astype(f32).reshape(4, P).T),
        "bv_row": g["bv"].astype(BF)[None, :],
        "bo_tok": np.tile(g["bo"].astype(f32), (P, 1)),
        "b1_c": np.ascontiguousarray(g["b1"].astype(f32).reshape(16, P).T),
        "b2_c": np.ascontiguousarray(g["b2"].astype(f32).reshape(4, P).T),
        "g1_tok": np.tile(g["ln1_g"].astype(f32), (P, 1)),
        "b1g_tok": np.tile(g["ln1_b"].astype(f32), (P, 1)),
        "g2_tok": np.tile(g["ln2_g"].astype(f32), (P, 1)),
        "b2g_tok": np.tile(g["ln2_b"].astype(f32), (P, 1)),
        "wd_diff_tok": np.tile((g["Wd"][1] - g["Wd"][0]).astype(f32), (P, 1)),
        "bddiff": np.full((P, 1), float(g["bd"][1]) - float(g["bd"][0]), f32),
        "ones_row": np.ones((1, T), BF),
    }

    cs = g["copy_seq"].astype(np.int64)          # [S, B]
    mm_ = g["mem_mask"].astype(bool)             # [B, S]
    outs = g["outs"].astype(f32)                 # [T, B, D]
    mem = g["mem"].astype(f32)                   # [S, B, D]

    maxcnt = 0
    for b in range(B):
        cnt = np.bincount(cs[:, b] // 512, minlength=NWIN).max()
        maxcnt = max(maxcnt, int(cnt))
    wslot = 32
    while wslot < maxcnt:
        wslot *= 2
    assert wslot <= P, "pathological copy_seq distribution"
    sp = 64 * wslot

    per_core = []
    for b in range(B):
        idx = cs[:, b]
        # compact sorted order: window-by-window runs; slot grid: window w
        # occupies slots [w*wslot, w*wslot + n_w)
        order = []
        s_pad = np.full(sp, -1, np.int64)
        pmat = np.zeros((S, sp), f32)
        for w in range(NWIN):
            sel = np.nonzero(idx // 512 == w)[0]
            o_w = len(order)
            for r, s_ in enumerate(sel):
                s_pad[w * wslot + r] = s_
                pmat[o_w + r, w * wslot + r] = 1.0
            order.extend(sel.tolist())
        order = np.asarray(order, np.int64)
        assert len(order) == S
        memc = mem[order, b, :]                      # [S, D] sorted
        maskrow = np.where(mm_[b, order], NEG, 0.0).astype(f32)
        oh = np.zeros((P, sp // P, 512), f32)
        for jg in np.nonzero(s_pad >= 0)[0]:
            w = jg // wslot
            vloc = int(idx[s_pad[jg]] - 512 * w)
            oh[jg % P, jg // P, vloc] = 1.0
        per_core.append({
            "outsT": np.ascontiguousarray(outs[:, b, :].T.astype(BF)),
            "outs_tok": np.ascontiguousarray(outs[:, b, :]),
            "memT": np.ascontiguousarray(memc.T.astype(BF)),
            "maskrow": maskrow[None, :].astype(BF),
            "pmat": pmat.astype(BF),
            "onehot": np.ascontiguousarray(oh.reshape(P, -1).astype(BF)),
        })
    return shared, per_core, wslot


def kernel(**inputs):
    shared, per_core, wslot = _prep(inputs)
    if wslot not in _CACHE:
        _CACHE[wslot] = _build(wslot)
    nc = _CACHE[wslot]
    in_maps = [{**shared, **pc} for pc in per_core]
    res = run_bass_kernel_spmd(nc, in_maps, core_ids=list(range(B)))
    return np.stack([r["out"] for r in res.results], axis=1)



# revision 2
# speedup vs baseline: 1.3075x; 1.3075x over previous
"""CopyTokenDecoder Trainium2 kernel (v2).

Sharding: data-parallel over batch B=8 -> one NeuronCore per batch element.

Key structure (per core):
  front-end: single-head attention, gating, FFN (bf16 weights).
  vocab: out[t,v] = log(gen_gate*softmax(logits)[t,v] + copy[t,v] + eps).
    For non-copy columns copy[t,v]=0 and eps is negligible, so
       out[t,v] = logits[t,v] + rowconst[t],
       rowconst = log(gen_gate) - log(sum_v exp(logits)).
    Pass A: fp8 DoubleRow matmuls -> PSUM logits -> exp (ACT, accum_out)
            gives the softmax denominator sv.  W_emb stays resident in SBUF
            as fp8 (128KB/partition).
    Pass B: recompute logits by matmul, add rowconst (DVE for t-tile 0,
            ACT Identity+bias for t-tile 1), store bf16.
  copy columns (<=512 unique vocab ids per batch) are handled in a compact
  [T,512] side path: delta = Mcol^T @ exp(scores) via matmul, compact logits
  via fp8 matmul against the host-gathered embedding rows, then
  log(exp(Lc+rowconst) + cg*rr*delta + eps); scattered into the full output
  on the host.  The attention row-sum rr is the row-sum of delta.
All ACT functions used (Exp/Ln/Square/Identity/Copy) live in one activation
table, so there are no table reloads.
"""

from contextlib import ExitStack

import numpy as np
import ml_dtypes

import concourse.tile as tile
from concourse import bacc, mybir
from concourse.bass_utils import run_bass_kernel_spmd
from concourse.masks import make_identity

F32 = mybir.dt.float32
BF16 = mybir.dt.bfloat16
FP8 = mybir.dt.float8e4
AF = mybir.ActivationFunctionType
OP = mybir.AluOpType
PM = mybir.MatmulPerfMode
BF = ml_dtypes.bfloat16
F8 = ml_dtypes.float8_e4m3

T, B, S, D, F, V = 256, 8, 512, 512, 2048, 32000
P = 128
DSCALE = float(D) ** -0.5
NEG = -1.0e30
TT = 2                      # t-tiles of 128
NCHUNK = 16                 # vocab chunks of 2048 (last covers 1280)
CHUNK = 2048
NCOL = 512                  # compact copy-column capacity
EPS_LN = 1e-5
EPS_LOG = 1e-12

_CACHE = {}


def _cw(c):
    return CHUNK if c < NCHUNK - 1 else V - (NCHUNK - 1) * CHUNK


def _subwidths(c):
    w = _cw(c)
    out = []
    while w > 0:
        out.append(min(512, w))
        w -= 512
    return out


def _build():
    nc = bacc.Bacc("TRN2", target_bir_lowering=False, debug=False,
                   enable_asserts=False, num_devices=B)

    def din(name, shape, dt):
        return nc.dram_tensor(name, shape, dt, kind="ExternalInput").ap()

    # per-core tensors
    outsT_d = din("outsT", [D, T], BF16)
    outs_tok_d = din("outs_tok", [T, D], F32)
    memT_d = din("memT", [D, S], BF16)
    maskrow_d = din("maskrow", [1, S], BF16)
    mcol_d = din("mcol", [S, NCOL], BF16)
    wcolsT_d = din("wcolsT", [D, NCOL], FP8)
    # shared weights
    wqT_d = din("wqT", [D, D], BF16)
    wkT_d = din("wkT", [D, D], BF16)
    wvT_d = din("wvT", [D, D], BF16)
    woT_d = din("woT", [D, D], BF16)
    w1T_d = din("w1T", [D, F], BF16)
    w2T_d = din("w2T", [F, D], BF16)
    wemb8_d = din("wemb8", [NCHUNK, P, 4 * CHUNK], FP8)
    bq_d = din("bq_c", [P, 4], F32)
    bk_d = din("bk_c", [P, 4], F32)
    bvrow_d = din("bv_row", [1, D], BF16)
    bo_tok_d = din("bo_tok", [P, D], F32)
    b1_d = din("b1_c", [P, 16], F32)
    b2_d = din("b2_c", [P, 4], F32)
    g1_d = din("g1_tok", [P, D], F32)
    b1g_d = din("b1g_tok", [P, D], F32)
    g2_d = din("g2_tok", [P, D], F32)
    b2g_d = din("b2g_tok", [P, D], F32)
    wdd_d = din("wd_diff_tok", [P, 2 * D], F32)
    nbdd_d = din("nbddiff", [P, 1], F32)
    ones_d = din("ones_row", [1, T], BF16)

    out_d = nc.dram_tensor("out", [T, V], BF16, kind="ExternalOutput").ap()
    out_r = out_d.rearrange("(th tl) v -> tl th v", tl=P)
    fix_d = nc.dram_tensor("fix", [T, NCOL], BF16, kind="ExternalOutput").ap()
    fix_r = fix_d.rearrange("(th tl) j -> tl th j", tl=P)

    r3 = lambda ap, inner: ap.rearrange("(hi lo) x -> lo hi x", lo=P)

    with tile.TileContext(nc) as tc, ExitStack() as octx:
        cpool = octx.enter_context(tc.tile_pool(name="cpool", bufs=1))
        # ---- persistent tiles ----
        ident_f = cpool.tile([P, P], F32, tag="ident_f")
        make_identity(nc, ident_f[:])
        ident_b = cpool.tile([P, P], BF16, tag="ident_b")
        nc.vector.tensor_copy(ident_b[:], ident_f[:])
        rr = cpool.tile([P, TT], F32, tag="rr")          # 1/sum_s exp(scores)
        cgrr = cpool.tile([P, TT], F32, tag="cgrr")      # cg * rr
        cg = cpool.tile([P, TT], F32, tag="cg")          # copy gate
        gg = cpool.tile([P, TT], F32, tag="gg")          # gen gate
        sv_parts = cpool.tile([P, TT * NCHUNK], F32, tag="sv_parts")
        svs = cpool.tile([P, TT], F32, tag="svs")
        rowconst = cpool.tile([P, TT], F32, tag="rowconst")
        x2T8 = cpool.tile([P, 4, T], FP8, tag="x2T8")
        delta_tok = cpool.tile([P, TT, NCOL], F32, tag="delta_tok")
        onesr = cpool.tile([1, T], BF16, tag="onesr")
        nc.sync.dma_start(onesr[:], ones_d[:])
        maskrow = cpool.tile([1, S], BF16, tag="maskrow")
        nc.sync.dma_start(maskrow[:], maskrow_d[:])
        eps_ln_c = cpool.tile([P, 1], F32, tag="eps_ln_c")
        nc.gpsimd.memset(eps_ln_c[:], EPS_LN)
        eps_log_c = cpool.tile([P, 1], F32, tag="eps_log_c")
        nc.gpsimd.memset(eps_log_c[:], EPS_LOG)
        mcol = cpool.tile([P, 4, NCOL], BF16, tag="mcol")
        nc.sync.dma_start(mcol[:], r3(mcol_d, NCOL))
        wcolsT = cpool.tile([P, 4, NCOL], FP8, tag="wcolsT")
        nc.sync.dma_start(wcolsT[:], r3(wcolsT_d, NCOL))

        # ================= front-end =================
        fctx = ExitStack()
        fe = fctx.enter_context(tc.tile_pool(name="fe", bufs=1))

        def load(pool, dram_ap, shape, tag):
            t_ = pool.tile(shape, dram_ap.dtype, tag=tag, name=tag)
            nc.sync.dma_start(t_[:], dram_ap)
            return t_

        outs_tok = load(fe, outs_tok_d.rearrange("(th tl) d -> tl th d", tl=P),
                        [P, TT, D], "outs_tok")
        bo_tok = load(fe, bo_tok_d, [P, D], "bo_tok")
        g1_tok = load(fe, g1_d, [P, D], "g1_tok")
        b1g_tok = load(fe, b1g_d, [P, D], "b1g_tok")
        g2_tok = load(fe, g2_d, [P, D], "g2_tok")
        b2g_tok = load(fe, b2g_d, [P, D], "b2g_tok")
        wd_diff = load(fe, wdd_d, [P, 2 * D], "wd_diff")
        nbddiff = load(fe, nbdd_d, [P, 1], "nbddiff")
        attn_tok = fe.tile([P, TT, D], F32, tag="attn_tok")

        def layer_norm(pool, scr_pool, dst, src_ap, g_t, b_t, nm):
            """dst[:] = LN(src_ap) * g + b   (token-major [P, D] slices).
            rstd computed as exp(-0.5*ln(ss/D+eps)) to stay on one ACT table."""
            mu = pool.tile([P, 1], F32, tag=f"mu_{nm}", name=f"mu_{nm}")
            nc.vector.reduce_sum(mu[:], src_ap, axis=mybir.AxisListType.X)
            nc.vector.tensor_scalar(out=mu[:], in0=mu[:], scalar1=1.0 / D,
                                    scalar2=None, op0=OP.mult)
            xc = scr_pool.tile([P, D], F32, tag="ln_xc", name="ln_xc")
            nc.vector.tensor_scalar(out=xc[:], in0=src_ap, scalar1=mu[:],
                                    scalar2=None, op0=OP.subtract)
            scr = scr_pool.tile([P, D], F32, tag="ln_scr", name="ln_scr")
            ss = pool.tile([P, 1], F32, tag=f"ss_{nm}", name=f"ss_{nm}")
            nc.scalar.activation(scr[:], xc[:], AF.Square, accum_out=ss[:])
            lnv = pool.tile([P, 1], F32, tag=f"lnv_{nm}", name=f"lnv_{nm}")
            nc.scalar.activation(lnv[:], ss[:], AF.Ln, bias=eps_ln_c[:, :1],
                                 scale=1.0 / D)
            rstd = pool.tile([P, 1], F32, tag=f"rstd_{nm}", name=f"rstd_{nm}")
            nc.scalar.activation(rstd[:], lnv[:], AF.Exp, scale=-0.5)
            nc.vector.scalar_tensor_tensor(out=dst, in0=xc[:], scalar=rstd[:],
                                           in1=g_t[:], op0=OP.mult, op1=OP.mult)
            nc.vector.tensor_add(dst, dst, b_t[:])

        # ---------------- stage A: attention ----------------
        with ExitStack() as actx:
            fa = actx.enter_context(tc.tile_pool(name="fa", bufs=1))
            fad = actx.enter_context(tc.tile_pool(name="fad", bufs=2))
            fp = actx.enter_context(tc.tile_pool(name="fp", bufs=6, space="PSUM"))
            fp5 = actx.enter_context(tc.tile_pool(name="fp5", bufs=2, space="PSUM"))

            outsT = load(fa, r3(outsT_d, T), [P, 4, T], "outsT")
            memT = load(fa, r3(memT_d, S), [P, 4, S], "memT")
            wqT = load(fa, r3(wqT_d, D), [P, 4, D], "wqT")
            wkT = load(fa, r3(wkT_d, D), [P, 4, D], "wkT")
            wvT = load(fa, r3(wvT_d, D), [P, 4, D], "wvT")
            woT = load(fa, r3(woT_d, D), [P, 4, D], "woT")
            bq_c = load(fa, bq_d, [P, 4], "bq_c")
            bk_c = load(fa, bk_d, [P, 4], "bk_c")
            bv_row = load(fa, bvrow_d, [1, D], "bv_row")

            # PE warm-up: junk matmuls gated only on on-chip data so they run
            # while the input DMAs land and the clock ramps.
            wu = fa.tile([P, 512], BF16, tag="wu")
            nc.gpsimd.memset(wu[:], 0.0)
            wu_ps = fp5.tile([P, 512], F32, tag="ps512", space="PSUM")
            for i in range(20):
                nc.tensor.matmul(wu_ps[:], lhsT=ident_b[:], rhs=wu[:],
                                 start=(i == 0), stop=(i == 19))

            # q/k projections (feature-major)
            qT = fa.tile([P, 4, T], BF16, tag="qT")
            for ho in range(4):
                ps = fp.tile([P, T], F32, tag="ps256", space="PSUM")
                for k in range(4):
                    nc.tensor.matmul(ps[:], lhsT=wqT[:, k, ho * P:(ho + 1) * P],
                                     rhs=outsT[:, k, :], start=(k == 0),
                                     stop=(k == 3))
                nc.vector.tensor_scalar(out=qT[:, ho, :], in0=ps[:],
                                        scalar1=bq_c[:, ho:ho + 1],
                                        scalar2=DSCALE, op0=OP.add, op1=OP.mult)
            kT = fa.tile([P, 4, S], BF16, tag="kT")
            for ho in range(4):
                ps = fp5.tile([P, 512], F32, tag="ps512", space="PSUM")
                for k in range(4):
                    nc.tensor.matmul(ps[:], lhsT=wkT[:, k, ho * P:(ho + 1) * P],
                                     rhs=memT[:, k, :], start=(k == 0),
                                     stop=(k == 3))
                nc.vector.tensor_scalar(out=kT[:, ho, :], in0=ps[:],
                                        scalar1=bk_c[:, ho:ho + 1],
                                        scalar2=None, op0=OP.add)
            # v (s-major)
            v_sb = fa.tile([P, 4, D], BF16, tag="v_sb")
            for sc in range(4):
                ps = fp5.tile([P, 512], F32, tag="ps512", space="PSUM")
                for k in range(4):
                    nc.tensor.matmul(ps[:], lhsT=memT[:, k, sc * P:(sc + 1) * P],
                                     rhs=wvT[:, k, :], start=(k == 0), stop=False)
                nc.tensor.matmul(ps[:], lhsT=onesr[:1, :P], rhs=bv_row[:],
                                 start=False, stop=True)
                if sc % 2 == 0:
                    nc.vector.tensor_copy(v_sb[:, sc, :], ps[:])
                else:
                    nc.scalar.copy(v_sb[:, sc, :], ps[:])

            # scoresT -> exp_c (s-major)
            exp_c = fa.tile([P, 4, T], BF16, tag="exp_c")
            for sc in range(4):
                ps = fp.tile([P, T], F32, tag="ps256", space="PSUM")
                for k in range(4):
                    nc.tensor.matmul(ps[:], lhsT=kT[:, k, sc * P:(sc + 1) * P],
                                     rhs=qT[:, k, :], start=(k == 0), stop=False)
                nc.tensor.matmul(ps[:], lhsT=maskrow[:1, sc * P:(sc + 1) * P],
                                 rhs=onesr[:1, :], start=False, stop=True)
                nc.scalar.activation(exp_c[:, sc, :], ps[:], AF.Exp)

            # compact copy delta: delta[t,j] = sum_s exp_c[s,t] * mcol[s,j]
            # row-sum of delta over j == full attention row sum (every memory
            # position maps to exactly one compact column).
            for tt in range(TT):
                ps = fp5.tile([P, 512], F32, tag="ps512", space="PSUM")
                for k in range(4):
                    nc.tensor.matmul(ps[:], lhsT=exp_c[:, k, tt * P:(tt + 1) * P],
                                     rhs=mcol[:, k, :], start=(k == 0),
                                     stop=(k == 3))
                nc.vector.reduce_sum(rr[:, tt:tt + 1], ps[:],
                                     axis=mybir.AxisListType.X)
                nc.scalar.copy(delta_tok[:, tt, :], ps[:])
            nc.vector.reciprocal(rr[:], rr[:])

            # attention value mix + output projection (feature-major)
            attnT = fa.tile([P, 4, T], BF16, tag="attnT")
            for dc in range(4):
                ps = fp.tile([P, T], F32, tag="ps256", space="PSUM")
                for sc in range(4):
                    nc.tensor.matmul(ps[:], lhsT=v_sb[:, sc, dc * P:(dc + 1) * P],
                                     rhs=exp_c[:, sc, :], start=(sc == 0),
                                     stop=(sc == 3))
                if dc % 2 == 0:
                    nc.vector.tensor_copy(attnT[:, dc, :], ps[:])
                else:
                    nc.scalar.copy(attnT[:, dc, :], ps[:])
            attn_oT = fa.tile([P, 4, T], F32, tag="attn_oT")
            for ho in range(4):
                ps = fp.tile([P, T], F32, tag="ps256", space="PSUM")
                for k in range(4):
                    nc.tensor.matmul(ps[:], lhsT=woT[:, k, ho * P:(ho + 1) * P],
                                     rhs=attnT[:, k, :], start=(k == 0),
                                     stop=(k == 3))
                if ho % 2 == 0:
                    nc.vector.tensor_copy(attn_oT[:, ho, :], ps[:])
                else:
                    nc.scalar.copy(attn_oT[:, ho, :], ps[:])

            # transpose to token-major; normalize rows; add bo
            for tt in range(TT):
                for ho in range(4):
                    pst = fp.tile([P, T], F32, tag="ps256", space="PSUM")
                    nc.tensor.transpose(pst[:, :P],
                                        attn_oT[:, ho, tt * P:(tt + 1) * P],
                                        ident_f[:])
                    if ho % 2 == 0:
                        nc.vector.tensor_copy(
                            attn_tok[:, tt, ho * P:(ho + 1) * P], pst[:, :P])
                    else:
                        nc.scalar.copy(attn_tok[:, tt, ho * P:(ho + 1) * P],
                                       pst[:, :P])
            for tt in range(TT):
                nc.vector.scalar_tensor_tensor(
                    out=attn_tok[:, tt, :], in0=attn_tok[:, tt, :],
                    scalar=rr[:, tt:tt + 1], in1=bo_tok[:],
                    op0=OP.mult, op1=OP.add)

        # ---------------- stage B: gates + FFN ----------------
        with ExitStack() as bctx:
            fb = bctx.enter_context(tc.tile_pool(name="fb", bufs=1))
            fbd = bctx.enter_context(tc.tile_pool(name="fbd", bufs=2))
            fp = bctx.enter_context(tc.tile_pool(name="fp2", bufs=8, space="PSUM"))

            w1T = load(fb, r3(w1T_d, F), [P, 4, F], "w1T")
            w2T = load(fb, r3(w2T_d, D), [P, 16, D], "w2T")
            b1_c = load(fb, b1_d, [P, 16], "b1_c")
            b2_c = load(fb, b2_d, [P, 4], "b2_c")

            # gates from [outs ; LN(attn)]: cg = sigmoid(ld + bddiff)
            # computed as 1/(1+exp(-ld - bddiff)) to avoid the sigmoid table.
            attn_n = fbd.tile([P, D], F32, tag="attn_n", name="attn_n")
            ld = fb.tile([P, TT], F32, tag="ld")
            for tt in range(TT):
                layer_norm(fb, fbd, attn_n[:], attn_tok[:, tt, :], g1_tok,
                           b1g_tok, "an")
                lda = fb.tile([P, 1], F32, tag="lda")
                ldb = fb.tile([P, 1], F32, tag="ldb")
                scr = fbd.tile([P, D], F32, tag="ld_scr", name="ld_scr")
                nc.vector.scalar_tensor_tensor(out=scr[:], in0=outs_tok[:, tt, :],
                                               scalar=1.0, in1=wd_diff[:, :D],
                                               op0=OP.mult, op1=OP.mult,
                                               accum_out=lda[:])
                scr2 = fbd.tile([P, D], F32, tag="ld_scr2", name="ld_scr2")
                nc.vector.scalar_tensor_tensor(out=scr2[:], in0=attn_n[:],
                                               scalar=1.0, in1=wd_diff[:, D:],
                                               op0=OP.mult, op1=OP.mult,
                                               accum_out=ldb[:])
                nc.vector.tensor_add(ld[:, tt:tt + 1], lda[:], ldb[:])
            et = fb.tile([P, TT], F32, tag="et")
            nc.scalar.activation(et[:], ld[:], AF.Exp, bias=nbddiff[:, :1],
                                 scale=-1.0)
            nc.vector.tensor_scalar(out=et[:], in0=et[:], scalar1=1.0,
                                    scalar2=None, op0=OP.add)
            nc.vector.reciprocal(cg[:], et[:])
            nc.vector.tensor_scalar(out=gg[:], in0=cg[:], scalar1=-1.0,
                                    scalar2=1.0, op0=OP.mult, op1=OP.add)
            nc.vector.tensor_mul(cgrr[:], cg[:], rr[:])

            # residual + LN1 -> x ; FFN ; LN2 -> x2 ; transpose -> x2T8 (fp8)
            x_tok = fb.tile([P, TT, D], F32, tag="x_tok")
            for tt in range(TT):
                res = fbd.tile([P, D], F32, tag="res", name="res")
                nc.vector.tensor_add(res[:], outs_tok[:, tt, :],
                                     attn_tok[:, tt, :])
                layer_norm(fb, fbd, x_tok[:, tt, :], res[:], g1_tok, b1g_tok, "x")
            xT = fb.tile([P, 4, T], BF16, tag="xT")
            for tt in range(TT):
                for k in range(4):
                    pst = fp.tile([P, T], F32, tag="ps256", space="PSUM")
                    nc.tensor.transpose(pst[:, :P],
                                        x_tok[:, tt, k * P:(k + 1) * P],
                                        ident_f[:])
                    if k % 2 == 0:
                        nc.vector.tensor_copy(xT[:, k, tt * P:(tt + 1) * P],
                                              pst[:, :P])
                    else:
                        nc.scalar.copy(xT[:, k, tt * P:(tt + 1) * P],
                                       pst[:, :P])
            h1T = fb.tile([P, 16, T], BF16, tag="h1T")
            for fc in range(16):
                ps = fp.tile([P, T], F32, tag="ps256", space="PSUM")
                for k in range(4):
                    nc.tensor.matmul(ps[:], lhsT=w1T[:, k, fc * P:(fc + 1) * P],
                                     rhs=xT[:, k, :], start=(k == 0),
                                     stop=(k == 3))
                nc.vector.tensor_scalar(out=h1T[:, fc, :], in0=ps[:],
                                        scalar1=b1_c[:, fc:fc + 1], scalar2=0.0,
                                        op0=OP.add, op1=OP.max)
            hT = fb.tile([P, 4, T], F32, tag="hT")
            for ho in range(4):
                ps = fp.tile([P, T], F32, tag="ps256", space="PSUM")
                for fc in range(16):
                    nc.tensor.matmul(ps[:], lhsT=w2T[:, fc, ho * P:(ho + 1) * P],
                                     rhs=h1T[:, fc, :], start=(fc == 0),
                                     stop=(fc == 15))
                nc.vector.tensor_scalar(out=hT[:, ho, :], in0=ps[:],
                                        scalar1=b2_c[:, ho:ho + 1], scalar2=None,
                                        op0=OP.add)
            h_tok = fb.tile([P, TT, D], F32, tag="h_tok")
            for tt in range(TT):
                for ho in range(4):
                    pst = fp.tile([P, T], F32, tag="ps256", space="PSUM")
                    nc.tensor.transpose(pst[:, :P],
                                        hT[:, ho, tt * P:(tt + 1) * P],
                                        ident_f[:])
                    if ho % 2 == 0:
                        nc.vector.tensor_copy(
                            h_tok[:, tt, ho * P:(ho + 1) * P], pst[:, :P])
                    else:
                        nc.scalar.copy(h_tok[:, tt, ho * P:(ho + 1) * P],
                                       pst[:, :P])
            x2_tok = fb.tile([P, TT, D], F32, tag="x2_tok")
            for tt in range(TT):
                layer_norm(fb, fbd, x2_tok[:, tt, :], h_tok[:, tt, :], g2_tok,
                           b2g_tok, "x2")
            for tt in range(TT):
                for k in range(4):
                    pst = fp.tile([P, T], F32, tag="ps256", space="PSUM")
                    nc.tensor.transpose(pst[:, :P],
                                        x2_tok[:, tt, k * P:(k + 1) * P],
                                        ident_f[:])
                    nc.scalar.copy(x2T8[:, k, tt * P:(tt + 1) * P], pst[:, :P])

        fctx.close()

        # ================= vocab passes =================
        bigp = octx.enter_context(tc.tile_pool(name="bigp", bufs=1))
        scrp = octx.enter_context(tc.tile_pool(name="scrp", bufs=2))
        outp = octx.enter_context(tc.tile_pool(name="outp", bufs=4))
        mp = octx.enter_context(tc.tile_pool(name="mp", bufs=2, space="PSUM"))

        # resident fp8 embedding: 16 x [P, 4, CHUNK] (128KB/partition)
        wembs = []
        for c in range(NCHUNK):
            w = bigp.tile([P, 4, CHUNK], FP8, tag=f"wemb{c}", name=f"wemb{c}")
            nc.sync.dma_start(w[:], wemb8_d[c].rearrange("p (hi v) -> p hi v",
                                                         v=CHUNK))
            wembs.append(w)

        def vocab_mms(ps, c, tt):
            for k_sub, wk in enumerate(_subwidths(c)):
                pslice = ps[:, k_sub * 512:k_sub * 512 + wk]
                for i in range(2):
                    nc.tensor.matmul(
                        pslice,
                        lhsT=x2T8[:, 2 * i:2 * i + 2, tt * P:(tt + 1) * P],
                        rhs=wembs[c][:, 2 * i:2 * i + 2,
                                     k_sub * 512:k_sub * 512 + wk],
                        start=(i == 0), stop=(i == 1), perf_mode=PM.DoubleRow)

        # ---- pass A: logits -> exp -> row sums ----
        for c in range(NCHUNK):
            cw = _cw(c)
            for tt in range(TT):
                ps = mp.tile([P, CHUNK], F32, tag="bigps", space="PSUM")
                vocab_mms(ps, c, tt)
                scr = scrp.tile([P, CHUNK], BF16, tag="escr", name="escr")
                nc.scalar.activation(
                    scr[:, :cw], ps[:, :cw], AF.Exp,
                    accum_out=sv_parts[:, tt * NCHUNK + c:tt * NCHUNK + c + 1])

        # ---- rowconst = ln(gg) - ln(sv) ----
        for tt in range(TT):
            nc.vector.reduce_sum(svs[:, tt:tt + 1],
                                 sv_parts[:, tt * NCHUNK:(tt + 1) * NCHUNK],
                                 axis=mybir.AxisListType.X)
        lgg = cpool.tile([P, TT], F32, tag="lgg")
        nc.scalar.activation(lgg[:], gg[:], AF.Ln)
        lsv = cpool.tile([P, TT], F32, tag="lsv")
        nc.scalar.activation(lsv[:], svs[:], AF.Ln)
        nc.vector.tensor_sub(rowconst[:], lgg[:], lsv[:])

        # ---- compact copy-column fixup ----
        fix_sb = cpool.tile([P, TT, NCOL], BF16, tag="fix_sb")
        for tt in range(TT):
            ps = mp.tile([P, CHUNK], F32, tag="bigps", space="PSUM")
            for i in range(2):
                nc.tensor.matmul(
                    ps[:, :NCOL],
                    lhsT=x2T8[:, 2 * i:2 * i + 2, tt * P:(tt + 1) * P],
                    rhs=wcolsT[:, 2 * i:2 * i + 2, :],
                    start=(i == 0), stop=(i == 1), perf_mode=PM.DoubleRow)
            ev = scrp.tile([P, NCOL], F32, tag="ev", name="ev")
            nc.scalar.activation(ev[:], ps[:, :NCOL], AF.Exp,
                                 bias=rowconst[:, tt:tt + 1])
            val = scrp.tile([P, NCOL], F32, tag="val", name="val")
            nc.vector.scalar_tensor_tensor(out=val[:], in0=delta_tok[:, tt, :],
                                           scalar=cgrr[:, tt:tt + 1], in1=ev[:],
                                           op0=OP.mult, op1=OP.add)
            nc.scalar.activation(fix_sb[:, tt, :], val[:], AF.Ln,
                                 bias=eps_log_c[:, :1])
            nc.sync.dma_start(fix_r[:, tt, :], fix_sb[:, tt, :])

        # ---- pass B: recompute logits, add rowconst, store bf16 ----
        for c in range(NCHUNK):
            cw = _cw(c)
            for tt in range(TT):
                ps = mp.tile([P, CHUNK], F32, tag="bigps", space="PSUM")
                vocab_mms(ps, c, tt)
                ot = outp.tile([P, CHUNK], BF16, tag="out_sb")
                if tt == 0:
                    nc.vector.tensor_scalar(out=ot[:, :cw], in0=ps[:, :cw],
                                            scalar1=rowconst[:, tt:tt + 1],
                                            scalar2=None, op0=OP.add)
                else:
                    nc.scalar.activation(ot[:, :cw], ps[:, :cw], AF.Identity,
                                         bias=rowconst[:, tt:tt + 1])
                nc.sync.dma_start(out_r[:, tt, c * CHUNK:c * CHUNK + cw],
                                  ot[:, :cw])

    nc.compile()
    return nc


def _tile_wemb8(w_emb):
    wp = np.zeros((NCHUNK * CHUNK, D), F8)
    wp[:V] = w_emb.astype(F8)
    # [c, v, hi, lo] -> [c, lo, hi, v]
    wt = wp.reshape(NCHUNK, CHUNK, 4, P).transpose(0, 3, 2, 1)
    return np.ascontiguousarray(wt.reshape(NCHUNK, P, 4 * CHUNK))


def _prep(inputs):
    g = {k: np.asarray(v) for k, v in inputs.items()}
    f32 = np.float32

    shared = {
        "wqT": np.ascontiguousarray(g["Wq"].T.astype(BF)),
        "wkT": np.ascontiguousarray(g["Wk"].T.astype(BF)),
        "wvT": np.ascontiguousarray(g["Wv"].T.astype(BF)),
        "woT": np.ascontiguousarray(g["Wo"].T.astype(BF)),
        "w1T": np.ascontiguousarray(g["W1"].T.astype(BF)),
        "w2T": np.ascontiguousarray(g["W2"].T.astype(BF)),
        "wemb8": _tile_wemb8(g["W_emb"]),
        "bq_c": np.ascontiguousarray(g["bq"].astype(f32).reshape(4, P).T),
        "bk_c": np.ascontiguousarray(g["bk"].astype(f32).reshape(4, P).T),
        "bv_row": g["bv"].astype(BF)[None, :],
        "bo_tok": np.tile(g["bo"].astype(f32), (P, 1)),
        "b1_c": np.ascontiguousarray(g["b1"].astype(f32).reshape(16, P).T),
        "b2_c": np.ascontiguousarray(g["b2"].astype(f32).reshape(4, P).T),
        "g1_tok": np.tile(g["ln1_g"].astype(f32), (P, 1)),
        "b1g_tok": np.tile(g["ln1_b"].astype(f32), (P, 1)),
        "g2_tok": np.tile(g["ln2_g"].astype(f32), (P, 1)),
        "b2g_tok": np.tile(g["ln2_b"].astype(f32), (P, 1)),
        "wd_diff_tok": np.tile((g["Wd"][1] - g["Wd"][0]).astype(f32), (P, 1)),
        "nbddiff": np.full((P, 1), -(float(g["bd"][1]) - float(g["bd"][0])),
                           f32),
        "ones_row": np.ones((1, T), BF),
    }

    cs = g["copy_seq"].astype(np.int64)          # [S, B]
    mm_ = g["mem_mask"].astype(bool)             # [B, S]
    outs = g["outs"].astype(f32)                 # [T, B, D]
    mem = g["mem"].astype(f32)                   # [S, B, D]
    w_emb = g["W_emb"].astype(f32)               # [V, D]

    per_core = []
    cols_list = []
    for b in range(B):
        idx = cs[:, b]
        cols, inv = np.unique(idx, return_inverse=True)
        ncols = len(cols)
        assert ncols <= NCOL
        mcol = np.zeros((S, NCOL), f32)
        mcol[np.arange(S), inv] = 1.0
        wcols = np.zeros((NCOL, D), f32)
        wcols[:ncols] = w_emb[cols]
        maskrow = np.where(mm_[b], NEG, 0.0).astype(f32)
        per_core.append({
            "outsT": np.ascontiguousarray(outs[:, b, :].T.astype(BF)),
            "outs_tok": np.ascontiguousarray(outs[:, b, :]),
            "memT": np.ascontiguousarray(mem[:, b, :].T.astype(BF)),
            "maskrow": maskrow[None, :].astype(BF),
            "mcol": mcol.astype(BF),
            "wcolsT": np.ascontiguousarray(wcols.T.astype(F8)),
        })
        cols_list.append(cols)
    return shared, per_core, cols_list


def kernel(**inputs):
    shared, per_core, cols_list = _prep(inputs)
    if "nc" not in _CACHE:
        _CACHE["nc"] = _build()
    nc = _CACHE["nc"]
    in_maps = [{**shared, **pc} for pc in per_core]
    res = run_bass_kernel_spmd(nc, in_maps, core_ids=list(range(B)))
    outs = []
    for b, r in enumerate(res.results):
        out = r["out"].astype(np.float32)        # [T, V]
        fix = r["fix"].astype(np.float32)        # [T, NCOL]
        cols = cols_list[b]
        out[:, cols] = fix[:, :len(cols)]
        outs.append(out)
    return np.stack(outs, axis=1)
